# revision 15
# baseline (speedup 1.0000x reference)
"""Trainium2 Bass kernel for DyDepthwiseConvAtten (v2 — vector conv).

Computation (per (b, n) row r of C=256 channels):
  w[r, k]  = sum_c q[r, c] * W_w[k, c] + b_w[k]          (k = 0..2)
  x[r, c]  = sum_k w[r, k] * vpad[r, c + k]               (3-tap depthwise conv, 'same')
  out[r,c] = (x - mean_c(x)) * rsqrt(var_c(x) + eps) * gamma[c] + beta[c]

Strategy: pure data-parallel over batch across 8 cores; rows (b*n
flattened) on SBUF partitions, tiles of 128 rows x 256 channels, G
row-tiles per DMA super-tile.

v1 ran the conv as diag(w_k) matmuls on PE — 10 PE instructions/tile
whose sequencer dispatch (~104 ns per matmul + ~60 ns per ldweights)
made PE.SEQ the bottleneck (~82 us of dispatch alone).  v2 moves the
conv to the vector engines as fused per-partition-scalar ops and leaves
PE only the two w-matmuls per tile:

  - w via TensorE: qT chunk [128c, 128r] stationary vs W_w^T [128c, 3],
    accumulated over the two c-chunks in PSUM (fp16 fast weight load).
  - conv via DVE on a zero-padded v tile [128, G, 258] (full-width ops,
    no edge cases):  t1 = v1 * w1 (tensor_scalar, 4x fp16 mode),
    t2 = v0 * w0 + t1, x = v2 * w2 + t2 (scalar_tensor_tensor, 2x), with
    accum_out on the last giving S1 = sum_c x for free.
  - S2 = sum_c x^2 on the otherwise-idle Pool engine (gpsimd):
    one scalar_tensor_tensor (x bypass) * x with accum_out.  Pool has no
    PSUM port, but x lives in SBUF fp16.
  - LN scalars batched per super-tile as [128, G] tiles: var*C^2 =
    C*S2 - S1^2 (2 DVE ops), rsqrt via ScalarE Sqrt(scale=1/C^2,
    bias=eps) + DVE reciprocal, nb = -S1/C * rs (1 DVE op).
  - normalize per tile on ScalarE: y = Id(x * rs + nb), fp16 out.

Per-tile engine budget (cost-model ns): DMA 549 (roofline) | DVE ~515 |
Act ~420 | PE ~330 | Pool ~250.  DMA-bound.

Accuracy (fp16 data path, fp32 stats/scalars): ~1e-3 normed rel error
vs the fp32 reference.
"""

import os
from contextlib import ExitStack

import numpy as np
import ml_dtypes

import concourse.bacc as bacc
import concourse.bass as bass
import concourse.tile as tile
from concourse import mybir
from concourse.bass_utils import run_bass_kernel_spmd

B, N, C, K = 1024, 100, 256, 3
N_CORES = 8
B_PER_CORE = B // N_CORES        # 128
ROWS = B_PER_CORE * N            # 12800 rows per core
P = 128                          # partitions (rows per tile)
N_ROW_TILES = ROWS // P          # 100
G = int(os.environ.get("BASS_DYCONV_G", "10"))  # row-tiles per DMA super-tile
assert N_ROW_TILES % G == 0
LN_EPS = 1e-5
F32 = mybir.dt.float32
FP16 = mybir.dt.float16

CONV_MODE = os.environ.get("BASS_DYCONV_MODE", "fp16o")
TRACE = bool(int(os.environ.get("BASS_DYCONV_TRACE", "0")))

LAST_EXEC_NS = None
LAST_RESULTS = None

_cache = {}


def _build(conv_mode: str, apply_affine: bool, add_bias: bool,
           loop_n: int = 1, ablate: str | None = None):
    global G
    G = int(os.environ.get("BASS_DYCONV_G", str(G)))
    assert N_ROW_TILES % G == 0
    nc = bacc.Bacc("TRN2", target_bir_lowering=False, debug=False)

    NS_ = N_ROW_TILES // G
    # qT: per-core transposed query, chunked: [2, 128c, ROWS]
    qT = nc.dram_tensor("qt", [2, P, ROWS], FP16, kind="ExternalInput")
    # v pre-permuted on host to the SBUF layout: [NS, p, g, c]
    v = nc.dram_tensor("v", [NS_, P, G, C], FP16, kind="ExternalInput")
    # W_w^T chunks: [2, 128c, K]
    wwt = nc.dram_tensor("wwt", [2, P, K], FP16, kind="ExternalInput")
    # out in [NS, p, g, c] layout; host inverse-permutes
    out = nc.dram_tensor("out", [NS_, P, G, C], FP16, kind="ExternalOutput")
    gamma = beta = bwb = None
    if apply_affine:
        gamma = nc.dram_tensor("gamma", [1, C], F32, kind="ExternalInput")
        beta = nc.dram_tensor("beta", [1, C], F32, kind="ExternalInput")
    if add_bias:
        bwb = nc.dram_tensor("bw", [1, K], F32, kind="ExternalInput")

    def emit():
        _emit(ctx, tc, qT.ap(), v.ap(), wwt.ap(), out.ap(),
              gamma.ap() if gamma is not None else None,
              beta.ap() if beta is not None else None,
              bwb.ap() if bwb is not None else None,
              conv_mode, ablate)

    with tile.TileContext(nc) as tc, ExitStack() as ctx:
        if loop_n > 1:
            with tc.For_i(0, loop_n, 1):
                emit()
        else:
            emit()
    nc.compile()
    return nc


def _bcast_rows(ap: bass.AP, nrows: int) -> bass.AP:
    """DMA access pattern replicating a [1, F] DRAM tensor across partitions."""
    return bass.AP(tensor=ap.tensor, offset=ap.offset,
                   ap=[[0, nrows]] + list(ap.ap[1:]))


def _emit(ctx, tc, qT, v, wwt, out, gamma, beta, bwb, conv_mode,
          ablate=None):
    nc = tc.nc
    mult = mybir.AluOpType.mult
    add = mybir.AluOpType.add
    sub = mybir.AluOpType.subtract
    byp = mybir.AluOpType.bypass
    AF = mybir.ActivationFunctionType

    singles = ctx.enter_context(tc.tile_pool(name="singles", bufs=1))
    pool = ctx.enter_context(tc.tile_pool(
        name="work", bufs=int(os.environ.get("BASS_DYCONV_WBUFS", "3"))))
    work2 = ctx.enter_context(tc.tile_pool(
        name="work2", bufs=int(os.environ.get("BASS_DYCONV_TBUFS", "2"))))
    small = ctx.enter_context(tc.tile_pool(
        name="small", bufs=int(os.environ.get("BASS_DYCONV_SMBUFS", "16"))))
    psum_w = ctx.enter_context(tc.tile_pool(
        name="psum_w", bufs=int(os.environ.get("BASS_DYCONV_WBUFSP", "4")),
        space=bass.MemorySpace.PSUM))

    # one-time constants
    wwt_sb = singles.tile([P, 2, K], FP16)
    nc.sync.dma_start(out=wwt_sb[:], in_=wwt.rearrange("a p k -> p a k"))
    eps_sb = singles.tile([P, 1], F32)
    nc.vector.memset(eps_sb[:], LN_EPS)
    if gamma is not None:
        gamma_sb = singles.tile([P, C], F32)
        nc.sync.dma_start(out=gamma_sb[:], in_=_bcast_rows(gamma, P))
        beta_sb = singles.tile([P, C], F32)
        nc.sync.dma_start(out=beta_sb[:], in_=_bcast_rows(beta, P))
    if bwb is not None:
        bw_sb = singles.tile([P, K], F32)
        nc.sync.dma_start(out=bw_sb[:], in_=_bcast_rows(bwb, P))

    CINV2 = 1.0 / (C * C)
    NS = N_ROW_TILES // G

    # one-time zeroing of the edge columns of every m0/m2 buffer (the tap
    # mults never write them; see emit_mults)
    tbufs = int(os.environ.get("BASS_DYCONV_TBUFS", "2"))
    for _ in range(tbufs):
        m0w = work2.tile([P, G, C], FP16, tag="m0")
        nc.vector.memset(m0w[:, :, 0:1], 0.0)
        m2w = work2.tile([P, G, C], FP16, tag="m2")
        nc.vector.memset(m2w[:, :, C - 1:C], 0.0)

    def start_loads(st):
        """Issue the DMA loads (and v-pad memsets) for super-tile st."""
        r0 = st * G * P
        qt_t = pool.tile([P, 2, G * P], FP16, tag="qt")
        nc.sync.dma_start(out=qt_t[:],
                          in_=qT[:, :, r0:r0 + G * P].rearrange(
                              "a c r -> c a r"))
        v_t = pool.tile([P, G, C], FP16, tag="vt")
        # split v across the two HWDGE queues (SP + Act) so the three
        # per-super-tile transfers balance at ~half the serial time each
        h = G // 2
        nc.sync.dma_start(out=v_t[:, 0:h, :], in_=v[st, :, 0:h])
        nc.scalar.dma_start(out=v_t[:, h:G, :], in_=v[st, :, h:G])
        return qt_t, v_t

    def emit_mults(st, qt_t, v_t):
        """PE w-matmuls + DVE tap-mults (with accum sums) -> m tiles, sm."""
        m0_t = work2.tile([P, G, C], FP16, tag="m0")
        m1_t = work2.tile([P, G, C], FP16, tag="m1")
        m2_t = work2.tile([P, G, C], FP16, tag="m2")
        a_t = work2.tile([P, G, C], FP16, tag="a")
        for g in range(G):
            if ablate == "now":
                w0 = w1 = w2 = eps_sb[:, 0:1]
            else:
                w_ps = psum_w.tile([P, K], F32, tag="w_ps")
                qt_g = qt_t[:, :, g * P:(g + 1) * P]
                nc.tensor.matmul(w_ps[:], lhsT=qt_g[:, 0, :],
                                 rhs=wwt_sb[:, 0, :], start=True,
                                 stop=False)
                nc.tensor.matmul(w_ps[:], lhsT=qt_g[:, 1, :],
                                 rhs=wwt_sb[:, 1, :], start=False,
                                 stop=True)
                if bwb is not None:
                    w_sb = small.tile([P, K], F32, tag="w_sb")
                    nc.vector.tensor_add(w_sb[:], w_ps[:], bw_sb[:])
                    w_ps = w_sb
                w0, w1, w2 = (w_ps[:, 0:1], w_ps[:, 1:2], w_ps[:, 2:3])

            v_g = v_t[:, g, :]
            # per-tap mults (tensor_scalar, 4x fp16 DVE mode).  v is
            # unpadded: the side taps write narrowed windows; the edge
            # columns m0[:,0] / m2[:,C-1] were zeroed once at startup and
            # are never written again.
            nc.vector.tensor_scalar(out=m1_t[:, g, :], in0=v_g[:, 0:C],
                                    scalar1=w1, scalar2=None, op0=mult)
            nc.vector.tensor_scalar(out=m0_t[:, g, 1:C], in0=v_g[:, 0:C - 1],
                                    scalar1=w0, scalar2=None, op0=mult)
            nc.vector.tensor_scalar(out=m2_t[:, g, 0:C - 1], in0=v_g[:, 1:C],
                                    scalar1=w2, scalar2=None, op0=mult)
        return (m0_t, m1_t, m2_t, a_t)

    def emit_adds(st, m):
        """Batched tap adds on the otherwise-idle Pool engine -> x_t."""
        m0_t, m1_t, m2_t, a_t = m
        x_t = pool.tile([P, G, C], FP16, tag="x")
        nc.vector.tensor_add(a_t[:], m0_t[:], m1_t[:])
        nc.gpsimd.tensor_add(x_t[:], a_t[:], m2_t[:])
        return x_t

    def emit_sq(st, x_t):
        """S1 = sum_c x, S2 = sum_c x^2 per sub-tile on ScalarE
        (Copy / Square with accum side-outputs)."""
        s1 = small.tile([P, G], F32, tag="s1")
        s2 = small.tile([P, G], F32, tag="s2")
        dump = work2.tile([P, G, C], FP16, tag="dump")
        dump2 = work2.tile([P, G, C], FP16, tag="dump2")
        for g in range(G):
            nc.scalar.activation(dump[:, g, :], x_t[:, g, :], AF.Square,
                                 accum_out=s2[:, g:g + 1])
            nc.scalar.activation(dump2[:, g, :], x_t[:, g, :], AF.Copy,
                                 accum_out=s1[:, g:g + 1])
        return s1, s2

    def emit_ln_norm(st, x_t, s1, s2):
        """Batched LN scalars + per-sub-tile normalize + store."""
        r0 = st * G * P
        msq = small.tile([P, G], F32, tag="msq")
        nc.vector.tensor_mul(msq[:], s1[:], s1[:])
        var_r = small.tile([P, G], F32, tag="var")  # C^2 * var
        nc.vector.scalar_tensor_tensor(out=var_r[:], in0=s2[:],
                                       scalar=float(C), in1=msq[:],
                                       op0=mult, op1=sub)
        rs = small.tile([P, G], F32, tag="rs")
        nc.scalar.activation(rs[:], var_r[:], AF.Sqrt, bias=eps_sb[:],
                             scale=CINV2)
        rsr = small.tile([P, G], F32, tag="rsr")
        nc.vector.reciprocal(rsr[:], rs[:])
        nb = small.tile([P, G], F32, tag="nb")
        nc.vector.scalar_tensor_tensor(out=nb[:], in0=s1[:],
                                       scalar=-1.0 / C, in1=rsr[:],
                                       op0=mult, op1=mult)
        y_t = pool.tile([P, G, C], FP16, tag="y")
        for g in range(G):
            y_g = y_t[:, g, :]
            nc.vector.tensor_scalar(out=y_g, in0=x_t[:, g, :],
                                    scalar1=rsr[:, g:g + 1],
                                    scalar2=nb[:, g:g + 1],
                                    op0=mult, op1=add)
            if gamma is not None:
                nc.vector.tensor_mul(y_g, y_g, gamma_sb[:])
                nc.vector.tensor_add(y_g, y_g, beta_sb[:])
        nc.scalar.dma_start(out=out[st], in_=y_t[:])

    def emit_copyout(st, src_t, col0=0):
        y_t = pool.tile([P, G, C], FP16, tag="y")
        for g in range(G):
            nc.scalar.copy(y_t[:, g, :], src_t[:, g, col0:col0 + C])
        nc.scalar.dma_start(out=out[st], in_=y_t[:])

    # software-pipelined: loads run one super-tile ahead, the LN/normalize
    # phase one behind, so no engine's in-order queue stalls on another's
    # same-super-tile work.
    loads = start_loads(0)
    pending = []  # [(st, x_t, sm, s2), ...] awaiting LN + normalize
    for st in range(NS):
        qt_t, v_t = loads
        if ablate == "dma":
            if st + 1 < NS:
                loads = start_loads(st + 1)
            emit_copyout(st, v_t, col0=0)
            continue
        m = emit_mults(st, qt_t, v_t)
        if st + 1 < NS:
            loads = start_loads(st + 1)
        x_t = emit_adds(st, m)
        if ablate == "noln":
            emit_copyout(st, x_t)
            continue
        s1, s2 = emit_sq(st, x_t)
        pending.append((st, x_t, s1, s2))
        if len(pending) > 2:
            emit_ln_norm(*pending.pop(0))
    for p in pending:
        emit_ln_norm(*p)


def kernel(query, value, W_w, b_w, gamma, beta):
    global LAST_EXEC_NS, LAST_RESULTS

    query = np.ascontiguousarray(np.asarray(query, dtype=np.float32))
    value = np.ascontiguousarray(np.asarray(value, dtype=np.float32))
    W_w = np.ascontiguousarray(np.asarray(W_w, dtype=np.float32))
    b_w = np.asarray(b_w, dtype=np.float32)
    gamma = np.asarray(gamma, dtype=np.float32)
    beta = np.asarray(beta, dtype=np.float32)

    apply_affine = not (np.all(gamma == 1.0) and np.all(beta == 0.0))
    add_bias = bool(np.any(b_w != 0.0))

    key = (CONV_MODE, apply_affine, add_bias)
    if key not in _cache:
        _cache[key] = _build(*key)
    nc = _cache[key]

    in_maps = host_in_maps(query, value, W_w, CONV_MODE)
    if apply_affine:
        for m in in_maps:
            m["gamma"] = gamma.reshape(1, C)
            m["beta"] = beta.reshape(1, C)
    if add_bias:
        for m in in_maps:
            m["bw"] = b_w.reshape(1, K)

    res = run_bass_kernel_spmd(nc, in_maps, core_ids=list(range(N_CORES)),
                               trace=TRACE)
    LAST_EXEC_NS = res.exec_time_ns
    LAST_RESULTS = res
    out = np.empty((B, N, C), dtype=np.float32)
    for c in range(N_CORES):
        out[c * B_PER_CORE:(c + 1) * B_PER_CORE] = (
            host_out_unpermute(res.results[c]["out"]).astype(
                np.float32).reshape(B_PER_CORE, N, C))
    return out


def host_in_maps(query, value, W_w, mode):
    """Shard + lay out inputs for the 8 cores (host-side, layout only)."""
    NS_ = N_ROW_TILES // G
    wwt = np.ascontiguousarray(W_w.T.reshape(2, P, K)).astype(np.float16)
    q_sh = query.reshape(N_CORES, ROWS, C)
    # v pre-permuted to the device layout [NS, p, g, c]
    v_sh = value.reshape(N_CORES, NS_, G, P, C).transpose(0, 1, 3, 2, 4)
    in_maps = []
    for c in range(N_CORES):
        in_maps.append({
            "qt": np.ascontiguousarray(q_sh[c].T).reshape(
                2, P, ROWS).astype(np.float16),
            "v": np.ascontiguousarray(v_sh[c].astype(np.float16)),
            "wwt": wwt,
        })
    return in_maps


def host_out_unpermute(o):
    """[NS, P, G, C] device layout -> [ROWS, C] row-major."""
    NS_ = N_ROW_TILES // G
    return o.reshape(NS_, P, G, C).transpose(0, 2, 1, 3).reshape(ROWS, C)


# revision 16
# speedup vs baseline: 1.0809x; 1.0809x over previous
"""Trainium2 Bass kernel for DyDepthwiseConvAtten (v2 — vector conv).

Computation (per (b, n) row r of C=256 channels):
  w[r, k]  = sum_c q[r, c] * W_w[k, c] + b_w[k]          (k = 0..2)
  x[r, c]  = sum_k w[r, k] * vpad[r, c + k]               (3-tap depthwise conv, 'same')
  out[r,c] = (x - mean_c(x)) * rsqrt(var_c(x) + eps) * gamma[c] + beta[c]

Strategy: pure data-parallel over batch across 8 cores; rows (b*n
flattened) on SBUF partitions, tiles of 128 rows x 256 channels, G
row-tiles per DMA super-tile.

v1 ran the conv as diag(w_k) matmuls on PE — 10 PE instructions/tile
whose sequencer dispatch (~104 ns per matmul + ~60 ns per ldweights)
made PE.SEQ the bottleneck (~82 us of dispatch alone).  v2 moves the
conv to the vector engines as fused per-partition-scalar ops and leaves
PE only the two w-matmuls per tile:

  - w via TensorE: qT chunk [128c, 128r] stationary vs W_w^T [128c, 3],
    accumulated over the two c-chunks in PSUM (fp16 fast weight load).
  - conv via DVE on a zero-padded v tile [128, G, 258] (full-width ops,
    no edge cases):  t1 = v1 * w1 (tensor_scalar, 4x fp16 mode),
    t2 = v0 * w0 + t1, x = v2 * w2 + t2 (scalar_tensor_tensor, 2x), with
    accum_out on the last giving S1 = sum_c x for free.
  - S2 = sum_c x^2 on the otherwise-idle Pool engine (gpsimd):
    one scalar_tensor_tensor (x bypass) * x with accum_out.  Pool has no
    PSUM port, but x lives in SBUF fp16.
  - LN scalars batched per super-tile as [128, G] tiles: var*C^2 =
    C*S2 - S1^2 (2 DVE ops), rsqrt via ScalarE Sqrt(scale=1/C^2,
    bias=eps) + DVE reciprocal, nb = -S1/C * rs (1 DVE op).
  - normalize per tile on ScalarE: y = Id(x * rs + nb), fp16 out.

Per-tile engine budget (cost-model ns): DMA 549 (roofline) | DVE ~515 |
Act ~420 | PE ~330 | Pool ~250.  DMA-bound.

Accuracy (fp16 data path, fp32 stats/scalars): ~1e-3 normed rel error
vs the fp32 reference.
"""

import os
from contextlib import ExitStack

import numpy as np
import ml_dtypes

import concourse.bacc as bacc
import concourse.bass as bass
import concourse.tile as tile
from concourse import mybir
from concourse.bass_utils import run_bass_kernel_spmd

B, N, C, K = 1024, 100, 256, 3
N_CORES = 8
B_PER_CORE = B // N_CORES        # 128
ROWS = B_PER_CORE * N            # 12800 rows per core
P = 128                          # partitions (rows per tile)
N_ROW_TILES = ROWS // P          # 100
G = int(os.environ.get("BASS_DYCONV_G", "10"))  # row-tiles per DMA super-tile
assert N_ROW_TILES % G == 0
LN_EPS = 1e-5
F32 = mybir.dt.float32
FP16 = mybir.dt.float16

CONV_MODE = os.environ.get("BASS_DYCONV_MODE", "fp16o")
TRACE = bool(int(os.environ.get("BASS_DYCONV_TRACE", "0")))

LAST_EXEC_NS = None
LAST_RESULTS = None

_cache = {}


def _build(conv_mode: str, apply_affine: bool, add_bias: bool,
           loop_n: int = 1, ablate: str | None = None):
    global G
    G = int(os.environ.get("BASS_DYCONV_G", str(G)))
    assert N_ROW_TILES % G == 0
    nc = bacc.Bacc("TRN2", target_bir_lowering=False, debug=False)

    NS_ = N_ROW_TILES // G
    # qT host-repacked per super-tile: [NS, 128c, 2, G*128r] (one
    # contiguous 5KB run per partition per DMA)
    qT = nc.dram_tensor("qt", [NS_, P, 2, G * P], FP16,
                        kind="ExternalInput")
    # v pre-permuted on host to the SBUF layout: [NS, p, g, c]
    v = nc.dram_tensor("v", [NS_, P, G, C], FP16, kind="ExternalInput")
    # W_w^T chunks: [2, 128c, K]
    wwt = nc.dram_tensor("wwt", [2, P, K], FP16, kind="ExternalInput")
    # out in [NS, p, g, c] layout; host inverse-permutes
    out = nc.dram_tensor("out", [NS_, P, G, C], FP16, kind="ExternalOutput")
    gamma = beta = bwb = None
    if apply_affine:
        gamma = nc.dram_tensor("gamma", [1, C], F32, kind="ExternalInput")
        beta = nc.dram_tensor("beta", [1, C], F32, kind="ExternalInput")
    if add_bias:
        bwb = nc.dram_tensor("bw", [1, K], F32, kind="ExternalInput")

    def emit():
        _emit(ctx, tc, qT.ap(), v.ap(), wwt.ap(), out.ap(),
              gamma.ap() if gamma is not None else None,
              beta.ap() if beta is not None else None,
              bwb.ap() if bwb is not None else None,
              conv_mode, ablate)

    with tile.TileContext(nc) as tc, ExitStack() as ctx:
        if loop_n > 1:
            with tc.For_i(0, loop_n, 1):
                emit()
        else:
            emit()
    nc.compile()
    return nc


def _bcast_rows(ap: bass.AP, nrows: int) -> bass.AP:
    """DMA access pattern replicating a [1, F] DRAM tensor across partitions."""
    return bass.AP(tensor=ap.tensor, offset=ap.offset,
                   ap=[[0, nrows]] + list(ap.ap[1:]))


def _emit(ctx, tc, qT, v, wwt, out, gamma, beta, bwb, conv_mode,
          ablate=None):
    nc = tc.nc
    mult = mybir.AluOpType.mult
    add = mybir.AluOpType.add
    sub = mybir.AluOpType.subtract
    byp = mybir.AluOpType.bypass
    AF = mybir.ActivationFunctionType

    singles = ctx.enter_context(tc.tile_pool(name="singles", bufs=1))
    pool = ctx.enter_context(tc.tile_pool(
        name="work", bufs=int(os.environ.get("BASS_DYCONV_WBUFS", "3"))))
    work2 = ctx.enter_context(tc.tile_pool(
        name="work2", bufs=int(os.environ.get("BASS_DYCONV_TBUFS", "2"))))
    small = ctx.enter_context(tc.tile_pool(
        name="small", bufs=int(os.environ.get("BASS_DYCONV_SMBUFS", "16"))))
    psum_w = ctx.enter_context(tc.tile_pool(
        name="psum_w", bufs=int(os.environ.get("BASS_DYCONV_WBUFSP", "4")),
        space=bass.MemorySpace.PSUM))

    # one-time constants
    wwt_sb = singles.tile([P, 2, K], FP16)
    nc.sync.dma_start(out=wwt_sb[:], in_=wwt.rearrange("a p k -> p a k"))
    eps_sb = singles.tile([P, 1], F32)
    nc.vector.memset(eps_sb[:], LN_EPS)
    if gamma is not None:
        gamma_sb = singles.tile([P, C], F32)
        nc.sync.dma_start(out=gamma_sb[:], in_=_bcast_rows(gamma, P))
        beta_sb = singles.tile([P, C], F32)
        nc.sync.dma_start(out=beta_sb[:], in_=_bcast_rows(beta, P))
    if bwb is not None:
        bw_sb = singles.tile([P, K], F32)
        nc.sync.dma_start(out=bw_sb[:], in_=_bcast_rows(bwb, P))

    CINV2 = 1.0 / (C * C)
    NS = N_ROW_TILES // G

    # one-time zeroing of the edge columns of every m0/m2 buffer (the tap
    # mults never write them; see emit_mults)
    tbufs = int(os.environ.get("BASS_DYCONV_TBUFS", "2"))
    for _ in range(tbufs):
        m0w = work2.tile([P, G, C], FP16, tag="m0")
        nc.vector.memset(m0w[:, :, 0:1], 0.0)
        m2w = work2.tile([P, G, C], FP16, tag="m2")
        nc.vector.memset(m2w[:, :, C - 1:C], 0.0)

    def start_loads(st):
        """Issue the DMA loads (and v-pad memsets) for super-tile st."""
        qt_t = pool.tile([P, 2, G * P], FP16, tag="qt")
        nc.sync.dma_start(out=qt_t[:], in_=qT[st])
        v_t = pool.tile([P, G, C], FP16, tag="vt")
        nc.sync.dma_start(out=v_t[:], in_=v[st])
        return qt_t, v_t

    def emit_mults(st, qt_t, v_t):
        """PE w-matmuls + DVE tap-mults (with accum sums) -> m tiles, sm."""
        m0_t = work2.tile([P, G, C], FP16, tag="m0")
        m1_t = work2.tile([P, G, C], FP16, tag="m1")
        m2_t = work2.tile([P, G, C], FP16, tag="m2")
        a_t = work2.tile([P, G, C], FP16, tag="a")
        for g in range(G):
            if ablate == "now":
                w0 = w1 = w2 = eps_sb[:, 0:1]
            else:
                w_ps = psum_w.tile([P, K], F32, tag="w_ps")
                qt_g = qt_t[:, :, g * P:(g + 1) * P]
                nc.tensor.matmul(w_ps[:], lhsT=qt_g[:, 0, :],
                                 rhs=wwt_sb[:, 0, :], start=True,
                                 stop=False)
                nc.tensor.matmul(w_ps[:], lhsT=qt_g[:, 1, :],
                                 rhs=wwt_sb[:, 1, :], start=False,
                                 stop=True)
                if bwb is not None:
                    w_sb = small.tile([P, K], F32, tag="w_sb")
                    nc.vector.tensor_add(w_sb[:], w_ps[:], bw_sb[:])
                    w_ps = w_sb
                w0, w1, w2 = (w_ps[:, 0:1], w_ps[:, 1:2], w_ps[:, 2:3])

            v_g = v_t[:, g, :]
            # per-tap mults (tensor_scalar, 4x fp16 DVE mode).  v is
            # unpadded: the side taps write narrowed windows; the edge
            # columns m0[:,0] / m2[:,C-1] were zeroed once at startup and
            # are never written again.
            nc.vector.tensor_scalar(out=m1_t[:, g, :], in0=v_g[:, 0:C],
                                    scalar1=w1, scalar2=None, op0=mult)
            nc.vector.tensor_scalar(out=m0_t[:, g, 1:C], in0=v_g[:, 0:C - 1],
                                    scalar1=w0, scalar2=None, op0=mult)
            nc.vector.tensor_scalar(out=m2_t[:, g, 0:C - 1], in0=v_g[:, 1:C],
                                    scalar1=w2, scalar2=None, op0=mult)
        return (m0_t, m1_t, m2_t, a_t)

    def emit_adds(st, m):
        """Batched tap adds on the otherwise-idle Pool engine -> x_t."""
        m0_t, m1_t, m2_t, a_t = m
        x_t = pool.tile([P, G, C], FP16, tag="x")
        nc.vector.tensor_add(a_t[:], m0_t[:], m1_t[:])
        nc.gpsimd.tensor_add(x_t[:], a_t[:], m2_t[:])
        return x_t

    def emit_sq(st, x_t):
        """S1 = sum_c x, S2 = sum_c x^2 per sub-tile on ScalarE
        (Copy / Square with accum side-outputs)."""
        s1 = small.tile([P, G], F32, tag="s1")
        s2 = small.tile([P, G], F32, tag="s2")
        dump = work2.tile([P, G, C], FP16, tag="dump")
        dump2 = work2.tile([P, G, C], FP16, tag="dump2")
        for g in range(G):
            nc.scalar.activation(dump[:, g, :], x_t[:, g, :], AF.Square,
                                 accum_out=s2[:, g:g + 1])
            nc.scalar.activation(dump2[:, g, :], x_t[:, g, :], AF.Copy,
                                 accum_out=s1[:, g:g + 1])
        return s1, s2

    def emit_ln_norm(st, x_t, s1, s2):
        """Batched LN scalars + per-sub-tile normalize + store."""
        r0 = st * G * P
        msq = small.tile([P, G], F32, tag="msq")
        nc.vector.tensor_mul(msq[:], s1[:], s1[:])
        var_r = small.tile([P, G], F32, tag="var")  # C^2 * var
        nc.vector.scalar_tensor_tensor(out=var_r[:], in0=s2[:],
                                       scalar=float(C), in1=msq[:],
                                       op0=mult, op1=sub)
        rs = small.tile([P, G], F32, tag="rs")
        nc.scalar.activation(rs[:], var_r[:], AF.Sqrt, bias=eps_sb[:],
                             scale=CINV2)
        rsr = small.tile([P, G], F32, tag="rsr")
        nc.vector.reciprocal(rsr[:], rs[:])
        nb = small.tile([P, G], F32, tag="nb")
        nc.vector.scalar_tensor_tensor(out=nb[:], in0=s1[:],
                                       scalar=-1.0 / C, in1=rsr[:],
                                       op0=mult, op1=mult)
        y_t = pool.tile([P, G, C], FP16, tag="y")
        for g in range(G):
            y_g = y_t[:, g, :]
            nc.vector.tensor_scalar(out=y_g, in0=x_t[:, g, :],
                                    scalar1=rsr[:, g:g + 1],
                                    scalar2=nb[:, g:g + 1],
                                    op0=mult, op1=add)
            if gamma is not None:
                nc.vector.tensor_mul(y_g, y_g, gamma_sb[:])
                nc.vector.tensor_add(y_g, y_g, beta_sb[:])
        nc.gpsimd.dma_start(out=out[st], in_=y_t[:])

    def emit_copyout(st, src_t, col0=0):
        y_t = pool.tile([P, G, C], FP16, tag="y")
        for g in range(G):
            nc.scalar.copy(y_t[:, g, :], src_t[:, g, col0:col0 + C])
        nc.scalar.dma_start(out=out[st], in_=y_t[:])

    # software-pipelined: loads run one super-tile ahead, the LN/normalize
    # phase one behind, so no engine's in-order queue stalls on another's
    # same-super-tile work.
    loads = start_loads(0)
    pending = []  # [(st, x_t, sm, s2), ...] awaiting LN + normalize
    for st in range(NS):
        qt_t, v_t = loads
        if ablate == "dma":
            if st + 1 < NS:
                loads = start_loads(st + 1)
            emit_copyout(st, v_t, col0=0)
            continue
        m = emit_mults(st, qt_t, v_t)
        if st + 1 < NS:
            loads = start_loads(st + 1)
        x_t = emit_adds(st, m)
        if ablate == "noln":
            emit_copyout(st, x_t)
            continue
        s1, s2 = emit_sq(st, x_t)
        pending.append((st, x_t, s1, s2))
        if len(pending) > 2:
            emit_ln_norm(*pending.pop(0))
    for p in pending:
        emit_ln_norm(*p)


def kernel(query, value, W_w, b_w, gamma, beta):
    global LAST_EXEC_NS, LAST_RESULTS

    query = np.ascontiguousarray(np.asarray(query, dtype=np.float32))
    value = np.ascontiguousarray(np.asarray(value, dtype=np.float32))
    W_w = np.ascontiguousarray(np.asarray(W_w, dtype=np.float32))
    b_w = np.asarray(b_w, dtype=np.float32)
    gamma = np.asarray(gamma, dtype=np.float32)
    beta = np.asarray(beta, dtype=np.float32)

    apply_affine = not (np.all(gamma == 1.0) and np.all(beta == 0.0))
    add_bias = bool(np.any(b_w != 0.0))

    key = (CONV_MODE, apply_affine, add_bias)
    if key not in _cache:
        _cache[key] = _build(*key)
    nc = _cache[key]

    in_maps = host_in_maps(query, value, W_w, CONV_MODE)
    if apply_affine:
        for m in in_maps:
            m["gamma"] = gamma.reshape(1, C)
            m["beta"] = beta.reshape(1, C)
    if add_bias:
        for m in in_maps:
            m["bw"] = b_w.reshape(1, K)

    res = run_bass_kernel_spmd(nc, in_maps, core_ids=list(range(N_CORES)),
                               trace=TRACE)
    LAST_EXEC_NS = res.exec_time_ns
    LAST_RESULTS = res
    out = np.empty((B, N, C), dtype=np.float32)
    for c in range(N_CORES):
        out[c * B_PER_CORE:(c + 1) * B_PER_CORE] = (
            host_out_unpermute(res.results[c]["out"]).astype(
                np.float32).reshape(B_PER_CORE, N, C))
    return out


def host_in_maps(query, value, W_w, mode):
    """Shard + lay out inputs for the 8 cores (host-side, layout only)."""
    NS_ = N_ROW_TILES // G
    wwt = np.ascontiguousarray(W_w.T.reshape(2, P, K)).astype(np.float16)
    q_sh = query.reshape(N_CORES, ROWS, C)
    # v pre-permuted to the device layout [NS, p, g, c]
    v_sh = value.reshape(N_CORES, NS_, G, P, C).transpose(0, 1, 3, 2, 4)
    in_maps = []
    for c in range(N_CORES):
        # qT [2, 128c, ROWS] -> [NS, 128c, 2, G*128]
        qt = q_sh[c].T.reshape(2, P, NS_, G * P).transpose(2, 1, 0, 3)
        in_maps.append({
            "qt": np.ascontiguousarray(qt).astype(np.float16),
            "v": np.ascontiguousarray(v_sh[c].astype(np.float16)),
            "wwt": wwt,
        })
    return in_maps


def host_out_unpermute(o):
    """[NS, P, G, C] device layout -> [ROWS, C] row-major."""
    NS_ = N_ROW_TILES // G
    return o.reshape(NS_, P, G, C).transpose(0, 2, 1, 3).reshape(ROWS, C)
